# revision 2
# baseline (speedup 1.0000x reference)
import sys
sys.path.insert(0, '/opt/trn_rl_repo')
import numpy as np

B = 16
H = 1024
W = 1024
K = 21
PAD = 10
NCORES = 8
WR = 148          # warp rows held per core (128 + 2*PAD)
HALF = 74
JCH = 32
NSTEP = 8
NGRP = 8
CPIX = HALF * JCH          # 2368 pixels per chunk
SLAB_R, SLAB_C = 48, 76
SLAB_E = SLAB_R * SLAB_C   # 3648
NI16 = CPIX // 16          # 148 idx cols per gather plane
LHW = 2 * K * 128          # 5376

_NC = None
LAST_EXEC_NS = None


def _build_nc():
    import concourse.bacc as bacc
    import concourse.mybir as mybir
    import concourse.tile as tile
    from contextlib import ExitStack

    f32 = mybir.dt.float32
    f32r = mybir.dt.float32r
    u16 = mybir.dt.uint16
    sub_op = mybir.AluOpType.subtract
    mul_op = mybir.AluOpType.mult
    add_op = mybir.AluOpType.add

    nc = bacc.Bacc()
    slab_d = nc.declare_dram_parameter("slab", [NSTEP, 128, SLAB_E], f32, isOutput=False)
    idx_d = nc.declare_dram_parameter("idx", [NSTEP, 128, 2 * NI16], u16, isOutput=False)
    wts_d = nc.declare_dram_parameter("wts", [NSTEP, 128, 2 * CPIX], f32, isOutput=False)
    lh_d = nc.declare_dram_parameter("lh", [128, LHW], f32r, isOutput=False)
    out_d = nc.declare_dram_parameter("out", [B, 128, W], f32, isOutput=True)

    with ExitStack() as ctx:
        tc = ctx.enter_context(tile.TileContext(nc))
        const = ctx.enter_context(tc.tile_pool(name="const", bufs=1))
        dpool = ctx.enter_context(tc.tile_pool(name="dsc", bufs=1, space="DRAM"))
        spool = ctx.enter_context(tc.tile_pool(name="slab", bufs=2))
        ipool = ctx.enter_context(tc.tile_pool(name="idx", bufs=2))
        wpool = ctx.enter_context(tc.tile_pool(name="wts", bufs=2))
        cpool = ctx.enter_context(tc.tile_pool(name="comb", bufs=2))
        gpool = ctx.enter_context(tc.tile_pool(name="gath", bufs=2))
        tpool = ctx.enter_context(tc.tile_pool(name="tmp", bufs=2))
        rpool = ctx.enter_context(tc.tile_pool(name="rhs", bufs=2))
        opool = ctx.enter_context(tc.tile_pool(name="ot", bufs=2))
        pspool = ctx.enter_context(tc.tile_pool(name="ps", bufs=2, space="PSUM"))

        scratch = dpool.tile([B, WR, W + 2 * PAD], f32r)

        lh_t = const.tile([128, LHW], f32r)
        nc.sync.dma_start(lh_t[:], lh_d[:, :])

        zt = const.tile([B, WR, PAD], f32)
        nc.vector.memset(zt[:], 0.0)
        nc.sync.dma_start(scratch[0:B, :, 0:PAD], zt[:].bitcast(f32r))
        nc.sync.dma_start(scratch[0:B, :, W + PAD:W + 2 * PAD], zt[:].bitcast(f32r))

        tt = nc.vector.tensor_tensor

        for s in range(NSTEP):
            slab_t = spool.tile([128, SLAB_E // 2, 2], f32)
            nc.sync.dma_start(slab_t[:], slab_d[s, :, :])
            idx_t = ipool.tile([128, 2 * NI16], u16)
            nc.sync.dma_start(idx_t[:], idx_d[s, :, :])
            wts_t = wpool.tile([128, 2 * CPIX], f32)
            nc.sync.dma_start(wts_t[:], wts_d[s, :, :])
            comb_t = cpool.tile([128, CPIX], f32)

            for off, ln in ((0, 1024), (1024, 1024), (2048, 320)):
                G0 = gpool.tile([128, 1024, 2], f32)
                G1 = gpool.tile([128, 1024, 2], f32)
                for q in range(0, ln, 512):
                    sz = min(512, ln - q)
                    o16 = (off + q) // 16
                    nc.gpsimd.indirect_copy(
                        G0[:, q:q + sz, :], slab_t[:], idx_t[:, o16:o16 + sz // 16],
                        i_know_ap_gather_is_preferred=True)
                    nc.gpsimd.indirect_copy(
                        G1[:, q:q + sz, :], slab_t[:],
                        idx_t[:, NI16 + o16:NI16 + o16 + sz // 16],
                        i_know_ap_gather_is_preferred=True)
                d_t = tpool.tile([128, 1024], f32)
                x1_t = tpool.tile([128, 1024], f32)
                g00 = G0[:, 0:ln, 0]
                g01 = G0[:, 0:ln, 1]
                g10 = G1[:, 0:ln, 0]
                g11 = G1[:, 0:ln, 1]
                cs = comb_t[:, off:off + ln]
                wxs = wts_t[:, off:off + ln]
                wys = wts_t[:, CPIX + off:CPIX + off + ln]
                dv = d_t[:, 0:ln]
                x1 = x1_t[:, 0:ln]
                tt(dv, g01, g00, op=sub_op)
                tt(dv, dv, wxs, op=mul_op)
                tt(cs, g00, dv, op=add_op)
                tt(dv, g11, g10, op=sub_op)
                tt(dv, dv, wxs, op=mul_op)
                tt(x1, g10, dv, op=add_op)
                tt(x1, x1, cs, op=sub_op)
                tt(x1, x1, wys, op=mul_op)
                tt(cs, cs, x1, op=add_op)

            for g in range(NGRP):
                h, jc = g // 4, 4 * s + (g % 4)
                nc.sync.dma_start(
                    scratch[0:B, HALF * h:HALF * h + HALF,
                            PAD + JCH * jc:PAD + JCH * jc + JCH],
                    comb_t[16 * g:16 * g + 16, :].bitcast(f32r))

        for img in range(B):
            for jh in range(2):
                rhs = rpool.tile([128, 2 * 532], f32r)
                nc.sync.dma_start(rhs[0:128, 0:532],
                                  scratch[img, 0:128, 512 * jh:512 * jh + 532])
                nc.sync.dma_start(rhs[0:20, 532:1064],
                                  scratch[img, 128:148, 512 * jh:512 * jh + 532])
                ps = pspool.tile([128, 512], mybir.dt.float32)
                for v in range(K):
                    nc.tensor.matmul(ps[:], lh_t[0:128, 128 * v:128 * v + 128],
                                     rhs[0:128, v:v + 512],
                                     start=(v == 0), stop=False)
                    nc.tensor.matmul(ps[:],
                                     lh_t[0:20, K * 128 + 128 * v:K * 128 + 128 * v + 128],
                                     rhs[0:20, 532 + v:532 + v + 512],
                                     start=False, stop=(v == K - 1))
                ot = opool.tile([128, 512], f32)
                nc.scalar.copy(ot[:], ps[:])
                nc.sync.dma_start(out_d[img, :, 512 * jh:512 * jh + 512], ot[:])

    nc.finalize()
    return nc


def _get_nc():
    global _NC
    if _NC is None:
        _NC = _build_nc()
    return _NC


def _geometry(x0, y0, raw_b, raw_rc, raw_subpix):
    b = np.log1p(np.exp(np.float64(raw_b))) + 1e-8
    rc = np.log1p(np.exp(np.float64(raw_rc))) + 1e-8
    sub = 0.25 * np.tanh(np.asarray(raw_subpix, np.float64))
    xs = np.linspace(-1.0, 1.0, W)
    ys = np.linspace(-1.0, 1.0, H)
    dx = xs - np.float64(x0)
    dy = ys - np.float64(y0)
    denom = np.sqrt(dx[:, None] ** 2 + dy[None, :] ** 2 + 1e-12 + rc * rc)
    gx = xs[:, None] - b * dx[:, None] / denom + sub[0]
    gy = ys[None, :] - b * dy[None, :] / denom + sub[1]
    ix = (gx + 1.0) * 0.5 * (W - 1)
    iy = (gy + 1.0) * 0.5 * (H - 1)
    ix0 = np.floor(ix).astype(np.int64)
    iy0 = np.floor(iy).astype(np.int64)
    wx = (ix - ix0).astype(np.float32)
    wy = (iy - iy0).astype(np.float32)
    assert ix0.min() >= 0 and ix0.max() + 1 <= W - 1
    assert iy0.min() >= 0 and iy0.max() + 1 <= H - 1
    return ix0, iy0, wx, wy


def _pack_core(c, srcn, ix0, iy0, wx, wy):
    rows = np.clip(np.arange(c * 128 - PAD, c * 128 - PAD + WR), 0, H - 1)
    IX0 = ix0[rows, :]
    IY0 = iy0[rows, :]
    WX = wx[rows, :]
    WY = wy[rows, :]
    slab = np.empty((NSTEP, 128, SLAB_E), np.float32)
    idxp = np.empty((NSTEP, 128, 2 * NI16), np.uint16)
    wts = np.empty((NSTEP, 128, 2 * CPIX), np.float32)
    for s in range(NSTEP):
        for g in range(NGRP):
            h, jc = g // 4, 4 * s + (g % 4)
            ksl = slice(HALF * h, HALF * h + HALF)
            jsl = slice(JCH * jc, JCH * jc + JCH)
            cy0 = IY0[ksl, jsl]
            cx0 = IX0[ksl, jsl]
            r0 = int(cy0.min())
            c0 = int(cx0.min())
            assert int(cy0.max()) + 1 - r0 <= SLAB_R - 1, "slab rows overflow"
            assert int(cx0.max()) + 1 - c0 <= SLAB_C - 1, "slab cols overflow"
            assert r0 + SLAB_R <= H and c0 + SLAB_C <= W
            slab[s, 16 * g:16 * g + 16] = \
                srcn[:, r0:r0 + SLAB_R, c0:c0 + SLAB_C].reshape(B, SLAB_E)
            fl0 = ((cy0 - r0) * SLAB_C + (cx0 - c0)).reshape(CPIX)
            idxp[s, 16 * g:16 * g + 16, 0:NI16] = \
                fl0.reshape(NI16, 16).T.astype(np.uint16)
            idxp[s, 16 * g:16 * g + 16, NI16:] = \
                (fl0 + SLAB_C).reshape(NI16, 16).T.astype(np.uint16)
            wts[s, 16 * g:16 * g + 16, 0:CPIX] = WX[ksl, jsl].reshape(1, CPIX)
            wts[s, 16 * g:16 * g + 16, CPIX:] = WY[ksl, jsl].reshape(1, CPIX)
    return slab, idxp, wts


def _pack_lh(c, psf):
    lh = np.zeros((128, LHW), np.float32)
    livek = (c * 128 - PAD + np.arange(128) >= 0) & (c * 128 - PAD + np.arange(128) < H)
    livek2 = (c * 128 + 118 + np.arange(20) >= 0) & (c * 128 + 118 + np.arange(20) < H)
    for v in range(K):
        for u in range(K):
            p = float(psf[u, v])
            ks = np.arange(u, 128)
            ms = np.arange(0, 128 - u)
            lh[ks, v * 128 + ms] = np.where(livek[ks], p, 0.0)
            ks2 = np.arange(0, 20)
            sel = ks2 + 1 <= u
            ks2 = ks2[sel]
            if ks2.size:
                ms2 = ks2 + 128 - u
                lh[ks2, K * 128 + v * 128 + ms2] = np.where(livek2[ks2], p, 0.0)
    return lh


def kernel(src, raw_psf, x0, y0, raw_b, raw_rc, raw_subpix):
    global LAST_EXEC_NS
    import time
    from concourse.bass_utils import run_bass_kernel_spmd

    srcn = np.asarray(src, np.float32).reshape(B, H, W)
    ix0, iy0, wx, wy = _geometry(float(x0), float(y0), float(raw_b), float(raw_rc),
                                 np.asarray(raw_subpix))

    psf = np.maximum(np.asarray(raw_psf, np.float64).reshape(K, K), 0.0)
    psf = psf / max(psf.sum(), 1e-12)
    psf = psf.astype(np.float32)

    in_maps = []
    for c in range(NCORES):
        slab, idxp, wts = _pack_core(c, srcn, ix0, iy0, wx, wy)
        in_maps.append({"slab": slab, "idx": idxp, "wts": wts,
                        "lh": _pack_lh(c, psf)})

    nc = _get_nc()
    t0 = time.perf_counter()
    res = run_bass_kernel_spmd(nc, in_maps, list(range(NCORES)))
    LAST_EXEC_NS = int((time.perf_counter() - t0) * 1e9)

    out = np.empty((B, 1, H, W), np.float32)
    for c in range(NCORES):
        out[:, 0, 128 * c:128 * c + 128, :] = res.results[c]["out"]
    return out
